# revision 8
# baseline (speedup 1.0000x reference)
"""ContextualContinuityLoss on 8 Trainium2 NeuronCores.

embeddings [32, 4096, 256] f32, labels [32, 4096] int -> scalar f32
loss = sum over (b, j) of mask(b,j) * ||e[b,j] - e[b,j+1]|| / (B*S),
mask = (l[b,j] != 0) & (l[b,j] == l[b,j+1]).

Data-parallel over the batch dim: 4 batches per core, each core emits its
partial (already divided by B*S); the host adds the 8 partial scalars.

Per-core layout: each batch row is padded host-side to S+1 positions (pad
row duplicates the last row, pad label = 0, so the fake pair contributes
exactly 0).  A tile covers 128*C consecutive pairs: partition p holds the
C+1 positions [base + p*C, base + p*C + C] (1-position overlap with the
next partition), so every pair's difference lives within one partition and
the D-reduction is a free-dim segmented reduce.
"""

import numpy as np

import concourse.bass as bass
import concourse.tile as tile
from concourse import bacc, bass_isa, mybir
from concourse.bass_utils import run_bass_kernel_spmd

B, S, D = 32, 4096, 256
N_CORES = 8
B_LOC = B // N_CORES  # batches per core
S_PAD = S + 1
P = 128  # partitions per tile
C = 16  # pairs per partition -> tile covers P*C = 2048 pairs
TILES_PER_BATCH = S // (P * C)  # 2
N_TILES = B_LOC * TILES_PER_BATCH  # 8

F32 = mybir.dt.float32
Alu = mybir.AluOpType


def build_kernel(b_loc=B_LOC, s=S, d=D, p=P, c=C, n_global=B * S):
    """Build the per-core Bass program. Shapes are per-core."""
    s_pad = s + 1
    tiles_per_batch = s // (p * c)
    assert s % (p * c) == 0
    n_tiles = b_loc * tiles_per_batch

    nc = bacc.Bacc("TRN2", target_bir_lowering=False, debug=False)
    emb = nc.dram_tensor("emb", [b_loc * s_pad, d], F32, kind="ExternalInput")
    lab = nc.dram_tensor("lab", [b_loc * s_pad], F32, kind="ExternalInput")
    out = nc.dram_tensor("out", [1, 1], F32, kind="ExternalOutput")

    with tile.TileContext(nc) as tc:
        with (
            tc.tile_pool(name="embp", bufs=3) as pe,
            tc.tile_pool(name="workp", bufs=2) as pw,
            tc.tile_pool(name="smallp", bufs=4) as ps,
            tc.tile_pool(name="accp", bufs=1) as pa,
        ):
            acc = pa.tile([p, n_tiles], F32)
            for b in range(b_loc):
                for t in range(tiles_per_batch):
                    idx = b * tiles_per_batch + t
                    base = b * s_pad + t * (p * c)

                    E = pe.tile([p, (c + 1) * d], F32)
                    src = bass.AP(
                        tensor=emb,
                        offset=base * d,
                        ap=[[c * d, p], [1, (c + 1) * d]],
                    )
                    nc.gpsimd.dma_start(out=E[:], in_=src)

                    L = ps.tile([p, c + 1], F32)
                    lsrc = bass.AP(
                        tensor=lab, offset=base, ap=[[c, p], [1, c + 1]]
                    )
                    nc.gpsimd.dma_start(out=L[:], in_=lsrc)

                    E3 = E[:].rearrange("q (k e) -> q k e", e=d)
                    DF = pw.tile([p, c * d], F32)
                    DF3 = DF[:].rearrange("q (k e) -> q k e", e=d)
                    nc.vector.tensor_sub(DF3, E3[:, 0:c, :], E3[:, 1 : c + 1, :])
                    nc.scalar.square(DF[:], DF[:])

                    SS = ps.tile([p, c], F32)
                    nc.vector.reduce_sum(out=SS[:], in_=DF3, axis=mybir.AxisListType.X)
                    NM = ps.tile([p, c], F32)
                    nc.scalar.sqrt(NM[:], SS[:])

                    # mask = (l0 == l1) * min(l0, 1)   with labels in {0..4}
                    EQ = ps.tile([p, c], F32)
                    nc.vector.tensor_tensor(
                        EQ[:], L[:, 0:c], L[:, 1 : c + 1], op=Alu.is_equal
                    )
                    MK = ps.tile([p, c], F32)
                    nc.vector.scalar_tensor_tensor(
                        out=MK[:],
                        in0=L[:, 0:c],
                        scalar=1.0,
                        in1=EQ[:],
                        op0=Alu.min,
                        op1=Alu.mult,
                    )

                    # masked = norm * mask ; acc[:, idx] = sum_free(masked)
                    # (tensor_tensor_reduce / ISA opcode 180 crashes this
                    # runtime's exec unit -- use two small DVE ops instead)
                    TR = ps.tile([p, c], F32)
                    nc.vector.tensor_mul(TR[:], NM[:], MK[:])
                    nc.vector.reduce_sum(
                        out=acc[:, idx : idx + 1],
                        in_=TR[:],
                        axis=mybir.AxisListType.X,
                    )

            total = pa.tile([p, 1], F32)
            nc.vector.reduce_sum(out=total[:], in_=acc[:], axis=mybir.AxisListType.X)
            # cross-partition: flip the column into a row via SBUF->SBUF DMA
            row = pa.tile([1, p], F32)
            nc.gpsimd.dma_start(out=row[0:1, :], in_=total[:, 0:1])
            res = pa.tile([1, 1], F32)
            nc.vector.reduce_sum(
                out=res[0:1, :], in_=row[0:1, :], axis=mybir.AxisListType.X
            )
            nc.scalar.mul(res[0:1, :], res[0:1, :], 1.0 / float(n_global))
            nc.gpsimd.dma_start(out=out.ap(), in_=res[0:1, 0:1])

    nc.compile()
    return nc


def prepare_core_inputs(embeddings: np.ndarray, labels: np.ndarray):
    """Shard batch over cores; pad each batch row to S+1 (dup last emb row,
    label 0) and flatten for the per-core DRAM layout."""
    emb = np.asarray(embeddings, dtype=np.float32)
    lab = np.asarray(labels).astype(np.float32)
    in_maps = []
    for core in range(N_CORES):
        eb = emb[core * B_LOC : (core + 1) * B_LOC]  # [B_LOC, S, D]
        lb = lab[core * B_LOC : (core + 1) * B_LOC]  # [B_LOC, S]
        eb_pad = np.concatenate([eb, eb[:, -1:, :]], axis=1)  # [B_LOC, S+1, D]
        lb_pad = np.concatenate(
            [lb, np.zeros((B_LOC, 1), np.float32)], axis=1
        )  # [B_LOC, S+1]
        in_maps.append(
            {
                "emb": np.ascontiguousarray(eb_pad.reshape(B_LOC * S_PAD, D)),
                "lab": np.ascontiguousarray(lb_pad.reshape(B_LOC * S_PAD)),
            }
        )
    return in_maps


_CACHE = {}


def _get_nc():
    if "nc" not in _CACHE:
        _CACHE["nc"] = build_kernel()
    return _CACHE["nc"]


def run_spmd(in_maps, **kwargs):
    return run_bass_kernel_spmd(
        _get_nc(), in_maps, core_ids=list(range(N_CORES)), **kwargs
    )


def kernel(embeddings: np.ndarray, labels: np.ndarray) -> np.ndarray:
    in_maps = prepare_core_inputs(embeddings, labels)
    res = run_spmd(in_maps)
    partials = [r["out"][0, 0] for r in res.results]
    return np.asarray(
        np.sum(np.asarray(partials, dtype=np.float64)), dtype=np.float32
    )


# revision 11
# speedup vs baseline: 1.1583x; 1.1583x over previous
"""ContextualContinuityLoss on 8 Trainium2 NeuronCores.

embeddings [32, 4096, 256] f32, labels [32, 4096] int -> scalar f32
loss = sum over (b, j) of mask(b,j) * ||e[b,j] - e[b,j+1]|| / (B*S),
mask = (l[b,j] != 0) & (l[b,j] == l[b,j+1]).

Data-parallel over the batch dim: 4 batches per core, each core emits its
partial (already divided by B*S); the host adds the 8 partial scalars.

Per-core layout: each batch row is padded host-side to S+1 positions (pad
row duplicates the last row, pad label = 0, so the fake pair contributes
exactly 0).  A tile covers 128*C consecutive pairs: partition p holds the
C+1 positions [base + p*C, base + p*C + C] (1-position overlap with the
next partition), so every pair's difference lives within one partition and
the D-reduction is a free-dim segmented reduce.

Embeddings travel as fp16: halves HBM traffic and lets the DVE run the
subtract and the reduce in its 16-bit 2x mode.  Only the pair differences
and their squares are fp16 (values O(10), no range issues); all sums are
fp32.  End-to-end error vs the fp32 reference is ~1e-4 relative.
"""

import numpy as np

import concourse.bass as bass
import concourse.tile as tile
from concourse import bacc, mybir
from concourse.bass_utils import run_bass_kernel_spmd

B, S, D = 32, 4096, 256
N_CORES = 8
B_LOC = B // N_CORES  # batches per core
S_PAD = S + 1
P = 128  # partitions per tile
C = 16  # pairs per partition -> tile covers P*C = 2048 pairs
TILES_PER_BATCH = S // (P * C)  # 2
N_TILES = B_LOC * TILES_PER_BATCH  # 8

F32 = mybir.dt.float32
F16 = mybir.dt.float16
Alu = mybir.AluOpType


def build_kernel(b_loc=B_LOC, s=S, d=D, p=P, c=C, n_global=B * S):
    """Build the per-core Bass program. Shapes are per-core."""
    s_pad = s + 1
    tiles_per_batch = s // (p * c)
    assert s % (p * c) == 0
    n_tiles = b_loc * tiles_per_batch

    nc = bacc.Bacc("TRN2", target_bir_lowering=False, debug=False)
    emb = nc.dram_tensor("emb", [b_loc * s_pad, d], F16, kind="ExternalInput")
    lab = nc.dram_tensor("lab", [b_loc * s_pad], F32, kind="ExternalInput")
    out = nc.dram_tensor("out", [1, 1], F32, kind="ExternalOutput")

    with tile.TileContext(nc) as tc:
        with (
            tc.tile_pool(name="embp", bufs=3) as pe,
            tc.tile_pool(name="workp", bufs=2) as pw,
            tc.tile_pool(name="smallp", bufs=1) as ps,
        ):
            # all labels in one DMA: [p, b, t, c+1] (partition chunks overlap
            # by one label, same as the embedding tiles)
            Lall = ps.tile([p, b_loc, tiles_per_batch, c + 1], F32)
            for b in range(b_loc):
                lsrc = bass.AP(
                    tensor=lab,
                    offset=b * s_pad,
                    ap=[[c, p], [p * c, tiles_per_batch], [1, c + 1]],
                )
                nc.gpsimd.dma_start(out=Lall[:, b, :, :], in_=lsrc)

            # per-pair sum of squared diffs, fp32, written tile by tile
            SQ = ps.tile([p, n_tiles, c], F32)

            for b in range(b_loc):
                for t in range(tiles_per_batch):
                    idx = b * tiles_per_batch + t
                    base = b * s_pad + t * (p * c)

                    E = pe.tile([p, (c + 1) * d], F16)
                    src = bass.AP(
                        tensor=emb,
                        offset=base * d,
                        ap=[[c * d, p], [1, (c + 1) * d]],
                    )
                    nc.gpsimd.dma_start(out=E[:], in_=src)

                    E3 = E[:].rearrange("q (k e) -> q k e", e=d)
                    DF = pw.tile([p, c * d], F16)
                    DF3 = DF[:].rearrange("q (k e) -> q k e", e=d)
                    nc.vector.tensor_sub(DF3, E3[:, 0:c, :], E3[:, 1 : c + 1, :])
                    nc.scalar.square(DF[:], DF[:])
                    nc.vector.reduce_sum(
                        out=SQ[:, idx, :], in_=DF3, axis=mybir.AxisListType.X
                    )

            # norms for every pair at once
            NM = ps.tile([p, n_tiles * c], F32)
            nc.scalar.sqrt(NM[:], SQ[:].rearrange("q a k -> q (a k)"))

            # mask = (l0 == l1) * min(l0, 1)   with labels in {0..4}
            L4 = Lall[:]
            l0 = L4[:, :, :, 0:c]
            l1 = L4[:, :, :, 1 : c + 1]
            EQ = ps.tile([p, b_loc, tiles_per_batch, c], F32)
            nc.vector.tensor_tensor(EQ[:], l0, l1, op=Alu.is_equal)
            MK = ps.tile([p, b_loc, tiles_per_batch, c], F32)
            nc.vector.scalar_tensor_tensor(
                out=MK[:],
                in0=l0,
                scalar=1.0,
                in1=EQ[:],
                op0=Alu.min,
                op1=Alu.mult,
            )

            # masked norms, then reduce everything to one value per partition
            TR = ps.tile([p, n_tiles * c], F32)
            nc.vector.tensor_mul(
                TR[:], NM[:], MK[:].rearrange("q b t k -> q (b t k)")
            )
            total = ps.tile([p, 1], F32)
            nc.vector.reduce_sum(out=total[:], in_=TR[:], axis=mybir.AxisListType.X)

            # cross-partition: flip the column into a row via SBUF->SBUF DMA
            row = ps.tile([1, p], F32)
            nc.gpsimd.dma_start(out=row[0:1, :], in_=total[:, 0:1])
            res = ps.tile([1, 1], F32)
            nc.vector.reduce_sum(
                out=res[0:1, :], in_=row[0:1, :], axis=mybir.AxisListType.X
            )
            nc.scalar.mul(res[0:1, :], res[0:1, :], 1.0 / float(n_global))
            nc.gpsimd.dma_start(out=out.ap(), in_=res[0:1, 0:1])

    nc.compile()
    return nc


def prepare_core_inputs(embeddings: np.ndarray, labels: np.ndarray):
    """Shard batch over cores; pad each batch row to S+1 (dup last emb row,
    label 0) and flatten for the per-core DRAM layout."""
    emb = np.asarray(embeddings, dtype=np.float32)
    lab = np.asarray(labels).astype(np.float32)
    in_maps = []
    for core in range(N_CORES):
        eb = emb[core * B_LOC : (core + 1) * B_LOC]  # [B_LOC, S, D]
        lb = lab[core * B_LOC : (core + 1) * B_LOC]  # [B_LOC, S]
        eb_pad = np.concatenate([eb, eb[:, -1:, :]], axis=1)  # [B_LOC, S+1, D]
        lb_pad = np.concatenate(
            [lb, np.zeros((B_LOC, 1), np.float32)], axis=1
        )  # [B_LOC, S+1]
        in_maps.append(
            {
                "emb": np.ascontiguousarray(
                    eb_pad.reshape(B_LOC * S_PAD, D).astype(np.float16)
                ),
                "lab": np.ascontiguousarray(lb_pad.reshape(B_LOC * S_PAD)),
            }
        )
    return in_maps


_CACHE = {}


def _get_nc():
    if "nc" not in _CACHE:
        _CACHE["nc"] = build_kernel()
    return _CACHE["nc"]


def run_spmd(in_maps, **kwargs):
    return run_bass_kernel_spmd(
        _get_nc(), in_maps, core_ids=list(range(N_CORES)), **kwargs
    )


def kernel(embeddings: np.ndarray, labels: np.ndarray) -> np.ndarray:
    in_maps = prepare_core_inputs(embeddings, labels)
    res = run_spmd(in_maps)
    partials = [r["out"][0, 0] for r in res.results]
    return np.asarray(
        np.sum(np.asarray(partials, dtype=np.float64)), dtype=np.float32
    )


# revision 14
# speedup vs baseline: 1.2972x; 1.1199x over previous
"""ContextualContinuityLoss on 8 Trainium2 NeuronCores.

embeddings [32, 4096, 256] f32, labels [32, 4096] int -> scalar f32
loss = sum over (b, j) of mask(b,j) * ||e[b,j] - e[b,j+1]|| / (B*S),
mask = (l[b,j] != 0) & (l[b,j] == l[b,j+1]).

Data-parallel over the batch dim: 4 batches per core, each core emits its
partial (already divided by B*S); the host adds the 8 partial scalars.

Per-core layout: each batch row is padded host-side to S+1 positions (pad
row duplicates the last row, pad label = 0, so the fake pair contributes
exactly 0).  A tile covers 128*C consecutive pairs: partition p holds the
C+1 positions [base + p*C, base + p*C + C] (1-position overlap with the
next partition), so every pair's difference lives within one partition and
the D-reduction is a free-dim segmented reduce.

Embeddings travel as fp16: halves HBM traffic and lets the DVE run the
subtract and the reduce in its 16-bit 2x mode.  Only the pair differences
and their squares are fp16 (values O(10), no range issues); all sums are
fp32.  End-to-end error vs the fp32 reference is ~1e-4 relative.
"""

import numpy as np

import concourse.bass as bass
import concourse.tile as tile
from concourse import bacc, mybir
from concourse.bass_utils import run_bass_kernel_spmd

B, S, D = 32, 4096, 256
N_CORES = 8
B_LOC = B // N_CORES  # batches per core
S_PAD = S + 1
P = 128  # partitions per tile
C = 16  # pairs per partition -> tile covers P*C = 2048 pairs
TILES_PER_BATCH = S // (P * C)  # 2
N_TILES = B_LOC * TILES_PER_BATCH  # 8

F32 = mybir.dt.float32
F16 = mybir.dt.float16
Alu = mybir.AluOpType


def build_kernel(b_loc=B_LOC, s=S, d=D, p=P, c=C, n_global=B * S):
    """Build the per-core Bass program. Shapes are per-core."""
    s_pad = s + 1
    tiles_per_batch = s // (p * c)
    assert s % (p * c) == 0
    n_tiles = b_loc * tiles_per_batch

    nc = bacc.Bacc("TRN2", target_bir_lowering=False, debug=False)
    emb = nc.dram_tensor("emb", [b_loc * s_pad, d], F16, kind="ExternalInput")
    lab = nc.dram_tensor("lab", [b_loc * s_pad], F32, kind="ExternalInput")
    out = nc.dram_tensor("out", [1, 1], F32, kind="ExternalOutput")

    with tile.TileContext(nc) as tc:
        with (
            tc.tile_pool(name="embp", bufs=3) as pe,
            tc.tile_pool(name="workp", bufs=2) as pw,
            tc.tile_pool(name="smallp", bufs=1) as ps,
        ):
            # all labels in one DMA: [p, b, t, c+1] (partition chunks overlap
            # by one label, same as the embedding tiles)
            Lall = ps.tile([p, b_loc, tiles_per_batch, c + 1], F32)
            for b in range(b_loc):
                lsrc = bass.AP(
                    tensor=lab,
                    offset=b * s_pad,
                    ap=[[c, p], [p * c, tiles_per_batch], [1, c + 1]],
                )
                nc.gpsimd.dma_start(out=Lall[:, b, :, :], in_=lsrc)

            # per-pair sum of squared diffs, fp32, written tile by tile
            SQ = ps.tile([p, n_tiles, c], F32)

            # Per tile the square+reduce work is split between engines to
            # balance their measured rates: the first ACT_FUSED pairs run as
            # fused ACT square+accum (square and D-reduction in one ~0.5us
            # instruction each); the rest get one contiguous ACT square plus
            # one DVE segmented reduce (reduce runs at 1x regardless of
            # dtype, ~0.27us/pair).
            act_fused = 4 if c > 8 else c // 2
            for b in range(b_loc):
                for t in range(tiles_per_batch):
                    idx = b * tiles_per_batch + t
                    base = b * s_pad + t * (p * c)

                    E = pe.tile([p, (c + 1) * d], F16)
                    src = bass.AP(
                        tensor=emb,
                        offset=base * d,
                        ap=[[c * d, p], [1, (c + 1) * d]],
                    )
                    nc.gpsimd.dma_start(out=E[:], in_=src)

                    E3 = E[:].rearrange("q (k e) -> q k e", e=d)
                    DF = pw.tile([p, c * d], F16)
                    DF3 = DF[:].rearrange("q (k e) -> q k e", e=d)
                    nc.vector.tensor_sub(DF3, E3[:, 0:c, :], E3[:, 1 : c + 1, :])
                    for i in range(act_fused):
                        nc.scalar.activation(
                            out=DF3[:, i, :],
                            in_=DF3[:, i, :],
                            func=mybir.ActivationFunctionType.Square,
                            accum_out=SQ[:, idx, i : i + 1],
                        )
                    if act_fused < c:
                        nc.scalar.square(
                            DF3[:, act_fused:c, :], DF3[:, act_fused:c, :]
                        )
                        nc.vector.reduce_sum(
                            out=SQ[:, idx, act_fused:c],
                            in_=DF3[:, act_fused:c, :],
                            axis=mybir.AxisListType.X,
                        )

            # norms for every pair at once
            NM = ps.tile([p, n_tiles * c], F32)
            nc.scalar.sqrt(NM[:], SQ[:].rearrange("q a k -> q (a k)"))

            # mask = (l0 == l1) * min(l0, 1)   with labels in {0..4}
            L4 = Lall[:]
            l0 = L4[:, :, :, 0:c]
            l1 = L4[:, :, :, 1 : c + 1]
            EQ = ps.tile([p, b_loc, tiles_per_batch, c], F32)
            nc.vector.tensor_tensor(EQ[:], l0, l1, op=Alu.is_equal)
            MK = ps.tile([p, b_loc, tiles_per_batch, c], F32)
            nc.vector.scalar_tensor_tensor(
                out=MK[:],
                in0=l0,
                scalar=1.0,
                in1=EQ[:],
                op0=Alu.min,
                op1=Alu.mult,
            )

            # masked norms, then reduce everything to one value per partition
            TR = ps.tile([p, n_tiles * c], F32)
            nc.vector.tensor_mul(
                TR[:], NM[:], MK[:].rearrange("q b t k -> q (b t k)")
            )
            total = ps.tile([p, 1], F32)
            nc.vector.reduce_sum(out=total[:], in_=TR[:], axis=mybir.AxisListType.X)

            # cross-partition: flip the column into a row via SBUF->SBUF DMA
            row = ps.tile([1, p], F32)
            nc.gpsimd.dma_start(out=row[0:1, :], in_=total[:, 0:1])
            res = ps.tile([1, 1], F32)
            nc.vector.reduce_sum(
                out=res[0:1, :], in_=row[0:1, :], axis=mybir.AxisListType.X
            )
            nc.scalar.mul(res[0:1, :], res[0:1, :], 1.0 / float(n_global))
            nc.gpsimd.dma_start(out=out.ap(), in_=res[0:1, 0:1])

    nc.compile()
    return nc


def prepare_core_inputs(embeddings: np.ndarray, labels: np.ndarray):
    """Shard batch over cores; pad each batch row to S+1 (dup last emb row,
    label 0) and flatten for the per-core DRAM layout."""
    emb = np.asarray(embeddings, dtype=np.float32)
    lab = np.asarray(labels).astype(np.float32)
    in_maps = []
    for core in range(N_CORES):
        eb = emb[core * B_LOC : (core + 1) * B_LOC]  # [B_LOC, S, D]
        lb = lab[core * B_LOC : (core + 1) * B_LOC]  # [B_LOC, S]
        eb_pad = np.concatenate([eb, eb[:, -1:, :]], axis=1)  # [B_LOC, S+1, D]
        lb_pad = np.concatenate(
            [lb, np.zeros((B_LOC, 1), np.float32)], axis=1
        )  # [B_LOC, S+1]
        in_maps.append(
            {
                "emb": np.ascontiguousarray(
                    eb_pad.reshape(B_LOC * S_PAD, D).astype(np.float16)
                ),
                "lab": np.ascontiguousarray(lb_pad.reshape(B_LOC * S_PAD)),
            }
        )
    return in_maps


_CACHE = {}


def _get_nc():
    if "nc" not in _CACHE:
        _CACHE["nc"] = build_kernel()
    return _CACHE["nc"]


def run_spmd(in_maps, **kwargs):
    return run_bass_kernel_spmd(
        _get_nc(), in_maps, core_ids=list(range(N_CORES)), **kwargs
    )


def kernel(embeddings: np.ndarray, labels: np.ndarray) -> np.ndarray:
    in_maps = prepare_core_inputs(embeddings, labels)
    res = run_spmd(in_maps)
    partials = [r["out"][0, 0] for r in res.results]
    return np.asarray(
        np.sum(np.asarray(partials, dtype=np.float64)), dtype=np.float32
    )


# revision 15
# speedup vs baseline: 1.3812x; 1.0648x over previous
"""ContextualContinuityLoss on 8 Trainium2 NeuronCores.

embeddings [32, 4096, 256] f32, labels [32, 4096] int -> scalar f32
loss = sum over (b, j) of mask(b,j) * ||e[b,j] - e[b,j+1]|| / (B*S),
mask = (l[b,j] != 0) & (l[b,j] == l[b,j+1]).

Data-parallel over the batch dim: 4 batches per core, each core emits its
partial (already divided by B*S); the host adds the 8 partial scalars.

Per-core layout: each batch row is padded host-side to S+1 positions (pad
row duplicates the last row, pad label = 0, so the fake pair contributes
exactly 0).  A tile covers 128*C consecutive pairs: partition p holds the
C+1 positions [base + p*C, base + p*C + C] (1-position overlap with the
next partition), so every pair's difference lives within one partition and
the D-reduction is a free-dim segmented reduce.

Embeddings travel as fp16: halves HBM traffic and lets the DVE run the
subtract and the reduce in its 16-bit 2x mode.  Only the pair differences
and their squares are fp16 (values O(10), no range issues); all sums are
fp32.  End-to-end error vs the fp32 reference is ~1e-4 relative.
"""

import numpy as np

import concourse.bass as bass
import concourse.tile as tile
from concourse import bacc, mybir
from concourse.bass_utils import run_bass_kernel_spmd

B, S, D = 32, 4096, 256
N_CORES = 8
B_LOC = B // N_CORES  # batches per core
S_PAD = S + 1
P = 128  # partitions per tile
C = 16  # pairs per partition -> tile covers P*C = 2048 pairs
TILES_PER_BATCH = S // (P * C)  # 2
N_TILES = B_LOC * TILES_PER_BATCH  # 8

F32 = mybir.dt.float32
F16 = mybir.dt.float16
Alu = mybir.AluOpType


def build_kernel(b_loc=B_LOC, s=S, d=D, p=P, c=C, n_global=B * S):
    """Build the per-core Bass program. Shapes are per-core."""
    s_pad = s + 1
    tiles_per_batch = s // (p * c)
    assert s % (p * c) == 0
    n_tiles = b_loc * tiles_per_batch

    nc = bacc.Bacc("TRN2", target_bir_lowering=False, debug=False)
    emb = nc.dram_tensor("emb", [b_loc * s_pad, d], F16, kind="ExternalInput")
    lab = nc.dram_tensor("lab", [b_loc * s_pad], F32, kind="ExternalInput")
    out = nc.dram_tensor("out", [1, 1], F32, kind="ExternalOutput")

    with tile.TileContext(nc) as tc:
        with (
            tc.tile_pool(name="embp", bufs=3) as pe,
            tc.tile_pool(name="workp", bufs=2) as pw,
            tc.tile_pool(name="smallp", bufs=1) as ps,
        ):
            # all labels in one DMA: [p, b, t, c+1] (partition chunks overlap
            # by one label, same as the embedding tiles)
            Lall = ps.tile([p, b_loc, tiles_per_batch, c + 1], F32)
            for b in range(b_loc):
                lsrc = bass.AP(
                    tensor=lab,
                    offset=b * s_pad,
                    ap=[[c, p], [p * c, tiles_per_batch], [1, c + 1]],
                )
                nc.gpsimd.dma_start(out=Lall[:, b, :, :], in_=lsrc)

            # per-pair sum of squared diffs, fp32, written tile by tile
            SQ = ps.tile([p, n_tiles, c], F32)

            # mask = (l0 == l1) * min(l0, 1)   with labels in {0..4}
            # (depends only on labels -> computed during the DMA ramp,
            # off the critical tail)
            L4 = Lall[:]
            l0 = L4[:, :, :, 0:c]
            l1 = L4[:, :, :, 1 : c + 1]
            EQ = ps.tile([p, b_loc, tiles_per_batch, c], F32)
            nc.vector.tensor_tensor(EQ[:], l0, l1, op=Alu.is_equal)
            MK = ps.tile([p, b_loc, tiles_per_batch, c], F32)
            nc.vector.scalar_tensor_tensor(
                out=MK[:],
                in0=l0,
                scalar=1.0,
                in1=EQ[:],
                op0=Alu.min,
                op1=Alu.mult,
            )

            # Per tile: DVE subtract (fp16 2x), ACT squares written in a
            # half-split layout (front half of each pair's D at [0, c*d/2),
            # back half at [c*d/2, c*d)), so the D-halves combine with one
            # CONTIGUOUS fp16 tensor_add (2x mode) before the 1x-rate
            # segmented reduce sees only half the elements.
            half = d // 2
            for b in range(b_loc):
                for t in range(tiles_per_batch):
                    idx = b * tiles_per_batch + t
                    base = b * s_pad + t * (p * c)

                    E = pe.tile([p, (c + 1) * d], F16)
                    src = bass.AP(
                        tensor=emb,
                        offset=base * d,
                        ap=[[c * d, p], [1, (c + 1) * d]],
                    )
                    nc.gpsimd.dma_start(out=E[:], in_=src)

                    E3 = E[:].rearrange("q (k e) -> q k e", e=d)
                    DF = pw.tile([p, c * d], F16)
                    DF3 = DF[:].rearrange("q (k e) -> q k e", e=d)
                    nc.vector.tensor_sub(DF3, E3[:, 0:c, :], E3[:, 1 : c + 1, :])

                    DF4 = DF[:].rearrange("q (k h e) -> q k h e", h=2, e=half)
                    SQH = pw.tile([p, c * d], F16)
                    SQH4 = SQH[:].rearrange("q (h k e) -> q k h e", h=2, e=half)
                    nc.scalar.activation(
                        out=SQH4,
                        in_=DF4,
                        func=mybir.ActivationFunctionType.Square,
                    )
                    HT = pw.tile([p, c * half], F16)
                    nc.vector.tensor_add(
                        HT[:],
                        SQH[:, 0 : c * half],
                        SQH[:, c * half : c * d],
                    )
                    nc.vector.reduce_sum(
                        out=SQ[:, idx, :],
                        in_=HT[:].rearrange("q (k e) -> q k e", e=half),
                        axis=mybir.AxisListType.X,
                    )

            # norms for every pair at once
            NM = ps.tile([p, n_tiles * c], F32)
            nc.scalar.sqrt(NM[:], SQ[:].rearrange("q a k -> q (a k)"))

            # masked norms, then reduce everything to one value per partition
            TR = ps.tile([p, n_tiles * c], F32)
            nc.vector.tensor_mul(
                TR[:], NM[:], MK[:].rearrange("q b t k -> q (b t k)")
            )
            total = ps.tile([p, 1], F32)
            nc.vector.reduce_sum(out=total[:], in_=TR[:], axis=mybir.AxisListType.X)

            # cross-partition: flip the column into a row via SBUF->SBUF DMA
            row = ps.tile([1, p], F32)
            nc.gpsimd.dma_start(out=row[0:1, :], in_=total[:, 0:1])
            res = ps.tile([1, 1], F32)
            nc.vector.reduce_sum(
                out=res[0:1, :], in_=row[0:1, :], axis=mybir.AxisListType.X
            )
            nc.scalar.mul(res[0:1, :], res[0:1, :], 1.0 / float(n_global))
            nc.gpsimd.dma_start(out=out.ap(), in_=res[0:1, 0:1])

    nc.compile()
    return nc


def prepare_core_inputs(embeddings: np.ndarray, labels: np.ndarray):
    """Shard batch over cores; pad each batch row to S+1 (dup last emb row,
    label 0) and flatten for the per-core DRAM layout."""
    emb = np.asarray(embeddings, dtype=np.float32)
    lab = np.asarray(labels).astype(np.float32)
    in_maps = []
    for core in range(N_CORES):
        eb = emb[core * B_LOC : (core + 1) * B_LOC]  # [B_LOC, S, D]
        lb = lab[core * B_LOC : (core + 1) * B_LOC]  # [B_LOC, S]
        eb_pad = np.concatenate([eb, eb[:, -1:, :]], axis=1)  # [B_LOC, S+1, D]
        lb_pad = np.concatenate(
            [lb, np.zeros((B_LOC, 1), np.float32)], axis=1
        )  # [B_LOC, S+1]
        in_maps.append(
            {
                "emb": np.ascontiguousarray(
                    eb_pad.reshape(B_LOC * S_PAD, D).astype(np.float16)
                ),
                "lab": np.ascontiguousarray(lb_pad.reshape(B_LOC * S_PAD)),
            }
        )
    return in_maps


_CACHE = {}


def _get_nc():
    if "nc" not in _CACHE:
        _CACHE["nc"] = build_kernel()
    return _CACHE["nc"]


def run_spmd(in_maps, **kwargs):
    return run_bass_kernel_spmd(
        _get_nc(), in_maps, core_ids=list(range(N_CORES)), **kwargs
    )


def kernel(embeddings: np.ndarray, labels: np.ndarray) -> np.ndarray:
    in_maps = prepare_core_inputs(embeddings, labels)
    res = run_spmd(in_maps)
    partials = [r["out"][0, 0] for r in res.results]
    return np.asarray(
        np.sum(np.asarray(partials, dtype=np.float64)), dtype=np.float32
    )
